# revision 4
# baseline (speedup 1.0000x reference)
"""LIF spiking-neuron forward kernel for Trainium2 (8 NeuronCores, data-parallel over neurons).

Computes, for x[B,N,T] and per-neuron params decay_m/decay_s/vth[N]:
    M_t = dm*(M_{t-1} + x_t);  S_t = ds*(S_{t-1} + x_t)
    E_t = dm*E_{t-1} + vth*o_{t-1}
    u_t = M_t - S_t - E_t - vth;  o_t = (u_t > 0)
returning the spike train o[B,N,T] (f32, bitwise-exact vs the f32 reference).

Sharding: neurons split across 8 cores (512 each).  Per core, rows are
(b, n) pairs laid out time-major in SBUF.  M/S use tensor_tensor_scan
(exact rounding match); the E/o feedback runs as a 127-step sequential
loop of fused DVE ops; spikes are thresholded in bulk at the end.
"""

import numpy as np

import concourse.bacc as bacc
import concourse.bass as bass
import concourse.mybir as mybir
import concourse.tile as tile
from concourse.bass_utils import run_bass_kernel_spmd

F32 = mybir.dt.float32
ALU = mybir.AluOpType

B, N, T = 64, 4096, 128
NCORES = 8
NLOC = N // NCORES          # 512 neurons per core
NH = NLOC // 128            # 4 neuron chunks of 128 (partition dim)

LAST_RESULTS = None         # test harness reads trace/exec info from here

_cached_program = None


def build_program() -> bass.Bass:
    nc = bacc.Bacc(None, target_bir_lowering=False)
    x_d = nc.declare_dram_parameter("x", [B, NLOC, T], F32, isOutput=False)
    dm_d = nc.declare_dram_parameter("decay_m", [NLOC], F32, isOutput=False)
    ds_d = nc.declare_dram_parameter("decay_s", [NLOC], F32, isOutput=False)
    vth_d = nc.declare_dram_parameter("vth", [NLOC], F32, isOutput=False)
    out_d = nc.declare_dram_parameter("out", [B, NLOC, T], F32, isOutput=True)

    with tile.TileContext(nc) as tc:
        with (
            tc.tile_pool(name="big", bufs=1) as bigp,
            tc.tile_pool(name="xin", bufs=4) as xp,
            tc.tile_pool(name="ms", bufs=4) as msp,
            tc.tile_pool(name="const", bufs=1) as cp,
        ):
            # R holds r=M-S, then u2 (in place), then spikes (in place).
            # free index = (b*NH + h)*T + t
            R = bigp.tile([128, B * NH * T], F32)
            Rv = R[:].rearrange("p (b h t) -> p b h t", b=B, h=NH, t=T)

            # per-neuron params: [128, NH] with partition = n%128, f = n//128
            dm_c = cp.tile([128, NH], F32)
            ds_c = cp.tile([128, NH], F32)
            vth_c = cp.tile([128, NH], F32)
            nc.sync.dma_start(dm_c[:], dm_d[:].rearrange("(h p) -> p h", p=128))
            nc.sync.dma_start(ds_c[:], ds_d[:].rearrange("(h p) -> p h", p=128))
            nc.sync.dma_start(vth_c[:], vth_d[:].rearrange("(h p) -> p h", p=128))

            # broadcast decay columns along time for the scan data1 operand
            dmB = cp.tile([128, NH * T], F32)
            dsB = cp.tile([128, NH * T], F32)
            dmBv = dmB[:].rearrange("p (h t) -> p h t", h=NH)
            dsBv = dsB[:].rearrange("p (h t) -> p h t", h=NH)
            for h in range(NH):
                nc.vector.tensor_copy(dmBv[:, h, :], dm_c[:, h : h + 1].broadcast_to([128, T]))
                nc.vector.tensor_copy(dsBv[:, h, :], ds_c[:, h : h + 1].broadcast_to([128, T]))

            # E state and e2 temp: free index = h*B + b
            E = cp.tile([128, NH * B], F32)
            e2 = cp.tile([128, NH * B], F32)
            Ev = E[:].rearrange("p (h b) -> p h b", h=NH)
            E_bh = E[:].rearrange("p (h b) -> p b h", h=NH)  # enumerates (b, h)
            e2v = e2[:].rearrange("p (h b) -> p h b", h=NH)
            nc.vector.memset(E[:], 0.0)

            # ---- phase 1: load x, scan M and S, r = M - S into R ----
            for b in range(B):
                xb = xp.tile([128, NH * T], F32, tag="xb")
                xbv = xb[:].rearrange("p (h t) -> p h t", h=NH)
                nc.sync.dma_start(xbv, x_d[b].rearrange("(h p) t -> p h t", p=128))
                for h in range(NH):
                    Mt = msp.tile([128, T], F32, tag="M")
                    St = msp.tile([128, T], F32, tag="S")
                    nc.vector.tensor_tensor_scan(
                        Mt[:], xbv[:, h, :], dmBv[:, h, :], 0.0,
                        op0=ALU.add, op1=ALU.mult,
                    )
                    nc.vector.tensor_tensor_scan(
                        St[:], xbv[:, h, :], dsBv[:, h, :], 0.0,
                        op0=ALU.add, op1=ALU.mult,
                    )
                    nc.gpsimd.tensor_tensor(
                        Rv[:, b, h, :], Mt[:], St[:], op=ALU.subtract
                    )

            # ---- phase 2: sequential E/o recurrence, u2 in place in R ----
            # col 0 already holds u2_0 = r_0 (E_0 = 0 exactly).
            for t in range(1, T):
                for h in range(NH):
                    # e2 = (u2_{t-1} > vth) * vth     (exact select)
                    nc.vector.tensor_scalar(
                        e2v[:, h, :], Rv[:, :, h, t - 1],
                        vth_c[:, h : h + 1], vth_c[:, h : h + 1],
                        op0=ALU.is_gt, op1=ALU.mult,
                    )
                for h in range(NH):
                    # E = (E * dm) + e2
                    nc.vector.scalar_tensor_tensor(
                        Ev[:, h, :], Ev[:, h, :], dm_c[:, h : h + 1], e2v[:, h, :],
                        op0=ALU.mult, op1=ALU.add,
                    )
                # u2_t = r_t - E   (in place in R)
                nc.vector.tensor_tensor(
                    Rv[:, :, :, t], Rv[:, :, :, t], E_bh, op=ALU.subtract
                )

            # ---- phase 3+4: threshold to spikes in place, DMA out ----
            for h in range(NH):
                eng = nc.gpsimd if h == 3 else nc.vector
                eng.tensor_scalar(
                    Rv[:, :, h, :], Rv[:, :, h, :],
                    vth_c[:, h : h + 1], None, op0=ALU.is_gt,
                )
                nc.sync.dma_start(
                    out_d[:, h * 128 : (h + 1) * 128, :].rearrange("b p t -> p b t"),
                    Rv[:, :, h, :],
                )
    nc.finalize()
    return nc


def kernel(x, decay_m, decay_s, vth):
    global _cached_program, LAST_RESULTS
    if _cached_program is None:
        _cached_program = build_program()
    nc = _cached_program

    in_maps = []
    for c in range(NCORES):
        sl = slice(c * NLOC, (c + 1) * NLOC)
        in_maps.append(
            {
                "x": np.ascontiguousarray(x[:, sl, :], dtype=np.float32),
                "decay_m": np.ascontiguousarray(decay_m[sl], dtype=np.float32),
                "decay_s": np.ascontiguousarray(decay_s[sl], dtype=np.float32),
                "vth": np.ascontiguousarray(vth[sl], dtype=np.float32),
            }
        )
    res = run_bass_kernel_spmd(nc, in_maps, core_ids=list(range(NCORES)))
    LAST_RESULTS = res
    out = np.empty((B, N, T), np.float32)
    for c in range(NCORES):
        out[:, c * NLOC : (c + 1) * NLOC, :] = res.results[c]["out"]
    return out
